# revision 2
# baseline (speedup 1.0000x reference)
"""Trainium2 Bass kernel for nn_GAT (3-layer GAT, 8 NeuronCores) — v2.

Restructured vs baseline:
- raw-ft table rows (no exp(a2) fold): block per head = [128 ft | ones |
  a2hi | a2lo | pad] (BLK=144 cols, row 1152 cols bf16 = 2304B).
- 32-wide dst col-group windows: one-hot m is [128,32]; ptm [32,128].
- batched dense matmuls (3 per k-tile, N=387/387/258), ones col via bias row.
- tile-major shared table + 5 grouped AllGathers per layer, separate table
  tensors per layer, dense(l+1) interleaved per tile after edge(l).
- elu via Scalar relu/exp (avoids slow MIN,BYPASS path).
"""
import numpy as np

from dataclasses import dataclass

import ml_dtypes

import concourse.bacc as bacc
import concourse.bass as bass
import concourse.mybir as mybir
import concourse.tile as tile

BF16 = mybir.dt.bfloat16
F32 = mybir.dt.float32
I16 = mybir.dt.int16
P = 128
AF = mybir.ActivationFunctionType
OP = mybir.AluOpType
SLOPE = 0.01
WIN = 32          # dst col-group window width
CPB = 8           # chunks per gather batch


@dataclass
class Cfg:
    N: int = 20000
    E: int = 320000
    IN: int = 512
    HID: int = 128
    H: int = 8
    C: int = 64
    NC: int = 8
    BLK: int = 144
    debug: bool = False

    @property
    def NSH(self):
        return self.N // self.NC      # 2500

    @property
    def NT(self):
        return (self.NSH + P - 1) // P    # 20

    @property
    def ROW01(self):
        return self.H * self.BLK      # 1152

    @property
    def ROWF(self):
        return 128

    @property
    def K0(self):
        return ((self.IN + 1 + P - 1) // P) * P   # 640

    @property
    def K1(self):
        return ((self.H * self.HID + 1 + P - 1) // P) * P   # 1152


def _bf(x):
    return np.asarray(x, dtype=np.float32).astype(ml_dtypes.bfloat16)


def _wrap16(idx_arr):
    """idx list (len mult of 16) -> [128, len//16] int16 (16-part wrap,
    replicated across the 8 Q7 groups)."""
    n = len(idx_arr)
    assert n % 16 == 0
    w = np.asarray(idx_arr, dtype=np.int16).reshape(n // 16, 16).T
    return np.tile(w, (8, 1))


def row_of(node, cfg):
    """Tile-major table row index (vectorized)."""
    node = np.asarray(node, dtype=np.int64)
    c = node // cfg.NSH
    j = node % cfg.NSH
    t = j // P
    r = j % P
    g = t // 4
    # groups 0..3: 4 full tiles each (512 rows/core); group 4: tiles 16-19
    # (3*128+68 = 452 rows/core)
    full = g * 4096 + c * 512 + (t % 4) * P + r
    last = 16384 + c * 452 + (t - 16) * P + r
    return np.where(g < 4, full, last).astype(np.int64)


def host_prep(cfg: Cfg, inputs: dict):
    N, E, H, HID, NC = cfg.N, cfg.E, cfg.H, cfg.HID, cfg.NC
    NSH, NT = cfg.NSH, cfg.NT
    src = np.asarray(inputs["src"]).astype(np.int64)
    dst = np.asarray(inputs["dst"]).astype(np.int64)

    NG = P // WIN  # 4 col-groups per tile
    # --- per (core, tile, group) edge lists (dst-sorted for determinism) ---
    core_of = dst // NSH
    tile_of = (dst % NSH) // P
    grp_of = ((dst % NSH) % P) // WIN
    order = np.argsort(dst, kind="stable")
    buckets = [[[[] for _ in range(NG)] for _ in range(NT)] for _ in range(NC)]
    for e in order:
        buckets[core_of[e]][tile_of[e]][grp_of[e]].append(e)

    # chunk schedule (uniform across cores): nch[t][j] = max over cores
    nch = [[0] * NG for _ in range(NT)]
    for t in range(NT):
        for j in range(NG):
            mx = max(len(buckets[c][t][j]) for c in range(NC))
            nch[t][j] = (mx + P - 1) // P

    # per tile: ordered chunk list [(j, k_within_group)] round-robin across
    # col-groups (consecutive matmuls hit different PE col-groups/banks),
    # batches of <= CPB
    chunks_t = []
    batches_t = []
    for t in range(NT):
        cl = []
        kmax = max(nch[t])
        for k in range(kmax):
            for j in range(NG):
                if k < nch[t][j]:
                    cl.append((j, k))
        chunks_t.append(cl)
        bl = []
        rem = len(cl)
        while rem > 0:
            take = min(CPB, rem)
            bl.append(take)
            rem -= take
        batches_t.append(bl)

    nch_total = sum(len(cl) for cl in chunks_t)
    idx_cols = 0
    for t in range(NT):
        for nb in batches_t[t]:
            idx_cols += (nb * P) // 16

    # ptm partition-packing: chunk (t, ci) -> local block index; each block
    # holds up to NG ptms at partition offsets 32*j (distinct j per block)
    pbblk_t = []     # per tile: list of local block index per chunk
    nblk_t = []      # per tile: number of blocks
    for t in range(NT):
        used = []    # list of sets of j
        blks = []
        for (j, k) in chunks_t[t]:
            if used and j not in used[-1]:
                used[-1].add(j)
                blks.append(len(used) - 1)
            else:
                used.append({j})
                blks.append(len(used) - 1)
        pbblk_t.append(blks)
        nblk_t.append(len(used))
    ch0_t = np.cumsum([0] + [len(cl) for cl in chunks_t]).tolist()
    blk0_t = np.cumsum([0] + nblk_t).tolist()

    meta = dict(nch=nch, chunks_t=chunks_t, batches_t=batches_t,
                nch_total=nch_total, idx_cols=idx_cols,
                pbblk_t=pbblk_t, nblk_t=nblk_t, ch0_t=ch0_t, blk0_t=blk0_t,
                nblk_total=sum(nblk_t))

    # --- dense weight packs (shared across cores) ---
    def pack_w(Wl, bl, K, nh, F):
        # cols (h, F+1): f<F -> W[:,h,f]; f=F -> ones col (via bias row);
        # bias row K-1: b[h, f], ones at (K-1, h*(F+1)+F)
        D = Wl.shape[-2]
        M = np.zeros((K, nh * (F + 1)), np.float32)
        for h in range(nh):
            Wh = Wl[h] if Wl.ndim == 3 else Wl
            bh = bl[h] if bl.ndim == 2 else bl
            M[:D, h * (F + 1):h * (F + 1) + F] = Wh
            M[K - 1, h * (F + 1):h * (F + 1) + F] = bh
            M[K - 1, h * (F + 1) + F] = 1.0
        kt = K // P
        ncol = M.shape[1]
        return _bf(M.reshape(kt, P, ncol).transpose(1, 0, 2).reshape(P, kt * ncol))

    def pack_wlr(W, b, al, alb, ar, arb, K):
        D = W.shape[-2]
        if W.ndim == 3:
            wl = np.einsum("hdf,hf->dh", W, al)
            wr = np.einsum("hdf,hf->dh", W, ar)
            cl = np.einsum("hf,hf->h", b, al) + alb
            cr = np.einsum("hf,hf->h", b, ar) + arb
        else:
            wl = (W @ al)[:, None]
            wr = (W @ ar)[:, None]
            cl = np.atleast_1d(b @ al + alb)
            cr = np.atleast_1d(b @ ar + arb)
        nh = wl.shape[1]
        M = np.zeros((K, 2 * nh), np.float32)
        M[:D, :nh] = wl
        M[:D, nh:] = wr
        M[K - 1, :nh] = cl
        M[K - 1, nh:] = cr
        kt = K // P
        return _bf(M.reshape(kt, P, 2 * nh).transpose(1, 0, 2).reshape(P, kt * 2 * nh))

    W0s = pack_w(inputs["W0"], inputs["b0"], cfg.K0, H, HID)
    W1s = pack_w(inputs["W1"], inputs["b1"], cfg.K1, H, HID)
    Wfs = pack_w(inputs["Wf"][None] if inputs["Wf"].ndim == 2 else inputs["Wf"],
                 inputs["bf"][None], cfg.K1, 1, cfg.C)
    WLR0 = pack_wlr(inputs["W0"], inputs["b0"], inputs["al0"], inputs["alb0"],
                    inputs["ar0"], inputs["arb0"], cfg.K0)
    WLR1 = pack_wlr(inputs["W1"], inputs["b1"], inputs["al1"], inputs["alb1"],
                    inputs["ar1"], inputs["arb1"], cfg.K1)
    WLRf = pack_wlr(inputs["Wf"], inputs["bf"], inputs["alf"], inputs["albf"],
                    inputs["arf"], inputs["arbf"], cfg.K1)

    eye_bf16 = _bf(np.eye(P))
    feats = np.asarray(inputs["features"], np.float32)

    in_maps = []
    for c in range(NC):
        idx_blocks = []
        # build per-chunk src rows + window cols
        srcs_all = np.zeros((nch_total, P), np.int64)
        wcol_all = np.full((nch_total, P), -1, np.int64)
        ci = 0
        for t in range(NT):
            for (j, k) in chunks_t[t]:
                el = buckets[c][t][j][k * P:(k + 1) * P]
                if len(el) > 0:
                    srcs_all[ci, :len(el)] = src[el]
                    wcol_all[ci, :len(el)] = ((dst[el] % NSH) % P) - j * WIN
                ci += 1
        rows_all = row_of(srcs_all.reshape(-1), cfg).reshape(nch_total, P)
        # gather idx blocks per batch
        ci = 0
        for t in range(NT):
            for nb in batches_t[t]:
                ni = nb * P
                idx_blocks.append(_wrap16(rows_all[ci:ci + nb].reshape(-1)))
                ci += nb
        idx_in = np.concatenate(idx_blocks, axis=1)

        # one-hots: m [128, (chunk, WIN)]; ptm [32, (chunk, 128)]
        d_ar = np.arange(WIN)
        m_all = (wcol_all[:, :, None] == d_ar[None, None, :])  # [ch, e, d]
        m_in = _bf(m_all.transpose(1, 0, 2).reshape(P, nch_total * WIN))
        pt_in = _bf(m_all.transpose(2, 0, 1).reshape(WIN, nch_total * P))

        xs = feats[c * NSH:(c + 1) * NSH]
        xT = np.zeros((cfg.K0, NSH), np.float32)
        xT[:cfg.IN] = xs.T
        xT[cfg.K0 - 1] = 1.0
        kt0 = cfg.K0 // P
        featT = _bf(xT.reshape(kt0, P, NSH).transpose(1, 0, 2).reshape(P, kt0 * NSH))

        in_maps.append(dict(
            featT=featT, W0s=W0s, W1s=W1s, Wfs=Wfs,
            WLR0=WLR0, WLR1=WLR1, WLRf=WLRf,
            idx=idx_in, m_oh=m_in, pt_oh=pt_in, eye_bf16=eye_bf16,
            onesrow=_bf(np.ones((1, NSH))),
        ))
    return in_maps, meta


def build_nc(cfg: Cfg, meta: dict):
    N, H, HID, C, NC = cfg.N, cfg.H, cfg.HID, cfg.C, cfg.NC
    NSH, NT, BLK = cfg.NSH, cfg.NT, cfg.BLK
    K0, K1 = cfg.K0, cfg.K1
    kt0, kt1 = K0 // P, K1 // P
    NG = P // WIN
    nch, chunks_t, batches_t = meta["nch"], meta["chunks_t"], meta["batches_t"]

    MAXCH = max(len(cl) for cl in meta["chunks_t"])

    nc = bacc.Bacc("TRN2", target_bir_lowering=False, debug=False,
                   num_devices=NC)

    # ---------------- I/O ----------------
    featT = nc.dram_tensor("featT", [P, kt0 * NSH], BF16, kind="ExternalInput")
    W0s = nc.dram_tensor("W0s", [P, kt0 * H * (HID + 1)], BF16, kind="ExternalInput")
    W1s = nc.dram_tensor("W1s", [P, kt1 * H * (HID + 1)], BF16, kind="ExternalInput")
    Wfs = nc.dram_tensor("Wfs", [P, kt1 * (C + 1)], BF16, kind="ExternalInput")
    WLR0 = nc.dram_tensor("WLR0", [P, kt0 * 2 * H], BF16, kind="ExternalInput")
    WLR1 = nc.dram_tensor("WLR1", [P, kt1 * 2 * H], BF16, kind="ExternalInput")
    WLRf = nc.dram_tensor("WLRf", [P, kt1 * 2], BF16, kind="ExternalInput")
    idx_t = nc.dram_tensor("idx", [P, meta["idx_cols"]], I16, kind="ExternalInput")
    m_oh_t = nc.dram_tensor("m_oh", [P, meta["nch_total"] * WIN], BF16,
                            kind="ExternalInput")
    pt_oh_t = nc.dram_tensor("pt_oh", [WIN, meta["nch_total"] * P], BF16,
                             kind="ExternalInput")
    eye_bf16_t = nc.dram_tensor("eye_bf16", [P, P], BF16, kind="ExternalInput")
    onesrow_t = nc.dram_tensor("onesrow", [1, NSH], BF16, kind="ExternalInput")
    out_t = nc.dram_tensor("out", [NSH, C], F32, kind="ExternalOutput")

    # internal DRAM
    agin = [nc.dram_tensor(f"agin{l}", [NSH, cfg.ROW01 if l < 2 else cfg.ROWF],
                           BF16, kind="Internal") for l in range(3)]
    tbl = [nc.dram_tensor(f"tbl{l}", [N, cfg.ROW01 if l < 2 else cfg.ROWF],
                          BF16, kind="Internal", addr_space="Shared")
           for l in range(3)]
    dbg = {}
    if cfg.debug:
        dbg["tbl0"] = nc.dram_tensor("dbg_tbl0", [N, cfg.ROW01], BF16,
                                     kind="ExternalOutput")
        dbg["x1"] = nc.dram_tensor("dbg_x1", [NSH, H * HID], BF16,
                                   kind="ExternalOutput")

    from contextlib import ExitStack
    with tile.TileContext(nc) as tc, ExitStack() as es:
        cpool = es.enter_context(tc.tile_pool(name="consts", bufs=1))
        xpool = es.enter_context(tc.tile_pool(name="xt", bufs=1))
        wpool = es.enter_context(tc.tile_pool(name="wrk", bufs=4))
        spool = es.enter_context(tc.tile_pool(name="sm", bufs=3))
        g8pool = es.enter_context(tc.tile_pool(name="g8", bufs=3))
        ohpool = es.enter_context(tc.tile_pool(name="oh", bufs=2))
        rpool = es.enter_context(tc.tile_pool(name="rows", bufs=2))
        apool = es.enter_context(tc.tile_pool(name="acc", bufs=2, space="PSUM"))
        auxp = es.enter_context(tc.tile_pool(name="aux", bufs=2, space="PSUM"))

        # ---- constants ----
        eyeb = cpool.tile([P, P], BF16)
        idxs = cpool.tile([P, meta["idx_cols"]], I16)
        w0 = cpool.tile([P, kt0 * H * (HID + 1)], BF16)
        w1 = cpool.tile([P, kt1 * H * (HID + 1)], BF16)
        wf = cpool.tile([P, kt1 * (C + 1)], BF16)
        wlr0 = cpool.tile([P, kt0 * 2 * H], BF16)
        wlr1 = cpool.tile([P, kt1 * 2 * H], BF16)
        wlrf = cpool.tile([P, kt1 * 2], BF16)
        for dst_ap, src_ap in [(eyeb, eye_bf16_t), (idxs, idx_t), (w0, W0s),
                               (w1, W1s), (wf, Wfs), (wlr0, WLR0),
                               (wlr1, WLR1), (wlrf, WLRf)]:
            nc.sync.dma_start(out=dst_ap[:], in_=src_ap[:])

        xt_a = xpool.tile([P, kt0 * NSH], BF16, tag="xta")
        nc.sync.dma_start(out=xt_a[:], in_=featT[:])
        xt_b = xpool.tile([P, kt1 * NSH], BF16, tag="xtb")
        # layer-1/2 input: k-tile 8 = [zeros ; ones at partition 127]
        nc.vector.memset(xt_b[:, (kt1 - 1) * NSH:], 0)
        nc.sync.dma_start(out=xt_b[P - 1:P, (kt1 - 1) * NSH:kt1 * NSH],
                          in_=onesrow_t[:])
        zz = cpool.tile([P, P], BF16)
        nc.vector.memset(zz[:], 0)
        neg1 = cpool.tile([P, 1], F32)
        nc.vector.memset(neg1[:], -1.0)
        ones_t = cpool.tile([P, H * HID], BF16)
        nc.vector.memset(ones_t[:], 1.0)

        def psum_zero(bank_ap, width):
            # zeroing matmul: sets has_written for the whole used region so
            # subsequent start=False matmuls accumulate; gives every later
            # matmul a RAW dep on this one (ordering under Tile's scheduler).
            nc.tensor.matmul(out=bank_ap[:, 0:width], lhsT=zz[:],
                             rhs=w1[:, 0:width], start=True, stop=False)
        # resident per-node a1 hi/lo [P, (t, 16)]: written by dense; per tile
        # rearranged into window-major a1vw [32, (t, j, 2*nh)] via PE selects
        a1v = cpool.tile([P, NT * 2 * H], BF16)
        a1vw = cpool.tile([WIN, NT * NG * 2 * H], BF16)

        def rows_of(t):
            return min(P, NSH - t * P)

        # ============ dense tile ============
        def dense_tile(layer, t):
            if layer == 0:
                xt, ws, wlr, kt, nh, F = xt_a, w0, wlr0, kt0, H, HID
            elif layer == 1:
                xt, ws, wlr, kt, nh, F = xt_b, w1, wlr1, kt1, H, HID
            else:
                xt, ws, wlr, kt, nh, F = xt_b, wf, wlrf, kt1, 1, C
            rows = rows_of(t)
            F1 = F + 1
            ncol = nh * F1
            accA = apool.tile([P, 3 * (HID + 1)], F32, tag="accA", space="PSUM")
            accB = apool.tile([P, 3 * (HID + 1)], F32, tag="accB", space="PSUM")
            accC = apool.tile([P, 2 * (HID + 1) + 16], F32, tag="accC",
                              space="PSUM")
            banksz = 3 * (HID + 1)  # 387
            a_off = 2 * (HID + 1)   # 258 in accC
            a_ap = accC[:, a_off:a_off + 2 * nh]
            psum_zero(accC, a_off + 2 * nh)
            for k in range(kt):
                lhs = xt[:, k * NSH + t * P: k * NSH + t * P + rows]
                rhs_base = k * ncol
                if nh == 8:
                    nc.tensor.matmul(out=accA[:rows, :banksz], lhsT=lhs,
                                     rhs=ws[:, rhs_base:rhs_base + banksz],
                                     start=(k == 0), stop=(k == kt - 1))
                    nc.tensor.matmul(out=accB[:rows, :banksz], lhsT=lhs,
                                     rhs=ws[:, rhs_base + banksz:rhs_base + 2 * banksz],
                                     start=(k == 0), stop=(k == kt - 1))
                    nc.tensor.matmul(out=accC[:rows, :2 * F1], lhsT=lhs,
                                     rhs=ws[:, rhs_base + 2 * banksz:rhs_base + ncol],
                                     start=False, stop=(k == kt - 1))
                else:
                    nc.tensor.matmul(out=accC[:rows, :F1], lhsT=lhs,
                                     rhs=ws[:, rhs_base:rhs_base + ncol],
                                     start=False, stop=(k == kt - 1))
                nc.tensor.matmul(out=a_ap[:rows, :], lhsT=lhs,
                                 rhs=wlr[:, k * 2 * nh:(k + 1) * 2 * nh],
                                 start=False, stop=(k == kt - 1))
            # ---- build table row ----
            rowW = cfg.ROW01 if layer < 2 else cfg.ROWF
            blk = BLK if layer < 2 else cfg.ROWF
            rowb = rpool.tile([P, rowW], BF16, tag="rowb" if layer < 2 else "rowbf")
            rview = rowb[:].rearrange("p (h b) -> p h b", b=blk)
            if nh == 8:
                nc.vector.tensor_copy(out=rview[:rows, 0:3, 0:F1],
                                      in_=accA[:rows].rearrange(
                                          "p (h b) -> p h b", b=F1))
                nc.vector.tensor_copy(out=rview[:rows, 3:6, 0:F1],
                                      in_=accB[:rows].rearrange(
                                          "p (h b) -> p h b", b=F1))
                nc.vector.tensor_copy(out=rview[:rows, 6:8, 0:F1],
                                      in_=accC[:rows, :2 * F1].rearrange(
                                          "p (h b) -> p h b", b=F1))
            else:
                nc.vector.tensor_copy(out=rowb[:rows, 0:F1],
                                      in_=accC[:rows, 0:F1])
            # a2 hi/lo at block cols F+1, F+2
            a2_ap = a_ap[:rows, nh:2 * nh]
            nc.vector.tensor_copy(out=rview[:rows, :nh, F1:F1 + 1],
                                  in_=a2_ap[:, :, None])
            nc.vector.tensor_tensor(out=rview[:rows, :nh, F1 + 1:F1 + 2],
                                    in0=a2_ap[:, :, None],
                                    in1=rview[:rows, :nh, F1:F1 + 1],
                                    op=OP.subtract)
            nc.scalar.dma_start(out=agin[layer][t * P:t * P + rows, :],
                                in_=rowb[:rows, :])
            # a1 hi/lo directly into resident a1v cols for this tile
            a1_ap = a_ap[:rows, 0:nh]
            c0 = t * 2 * H
            nc.vector.tensor_copy(out=a1v[:rows, c0:c0 + nh], in_=a1_ap)
            nc.vector.tensor_tensor(out=a1v[:rows, c0 + nh:c0 + 2 * nh],
                                    in0=a1_ap, in1=a1v[:rows, c0:c0 + nh],
                                    op=OP.subtract)

        def ag_group(layer, g):
            rowW = cfg.ROW01 if layer < 2 else cfg.ROWF
            if g < 4:
                i0, i1 = g * 512, (g + 1) * 512
                o0 = g * 4096
            else:
                i0, i1 = 2048, 2500
                o0 = 16384
            o1 = o0 + (i1 - i0) * NC
            nc.gpsimd.collective_compute(
                "AllGather", OP.bypass, replica_groups=[list(range(NC))],
                ins=[agin[layer][i0:i1, :]], outs=[tbl[layer][o0:o1, :]])

        # ============ edge tile ============
        # meta column offsets walked per layer
        def edge_tile(layer, t, offs):
            final = (layer == 2)
            nhl = 1 if final else H
            F = C if final else HID
            F1 = F + 1
            blk = cfg.ROWF if final else BLK
            rowW = cfg.ROWF if final else cfg.ROW01
            tb = tbl[layer]
            rows = rows_of(t)
            accA = apool.tile([P, 3 * (HID + 1)], F32, tag="accA", space="PSUM")
            accB = apool.tile([P, 3 * (HID + 1)], F32, tag="accB", space="PSUM")
            accC = apool.tile([P, 2 * (HID + 1) + 16], F32, tag="accC",
                              space="PSUM")
            if final:
                psum_zero(accC, F1)
            else:
                psum_zero(accA, 3 * F1)
                psum_zero(accB, 3 * F1)
                psum_zero(accC, 2 * F1)

            def acc_ap(h, pr=None):
                if final:
                    base, til = accC, 0
                elif h < 3:
                    base, til = accA, h
                elif h < 6:
                    base, til = accB, h - 3
                else:
                    base, til = accC, h - 6
                if pr is None:
                    return base[:, til * F1:(til + 1) * F1]
                p0, p1 = pr
                return base[p0:p1, til * F1:(til + 1) * F1]

            chunk_list = chunks_t[t]
            n_chunks = len(chunk_list)
            ch0 = meta["ch0_t"][t]
            mball = ohpool.tile([P, MAXCH * WIN], BF16, tag="mb")
            pball = ohpool.tile([WIN, MAXCH * P], BF16, tag="pb")
            nc.sync.dma_start(out=mball[:, :n_chunks * WIN],
                              in_=m_oh_t[:, ch0 * WIN:(ch0 + n_chunks) * WIN])
            nc.sync.dma_start(out=pball[:, :n_chunks * P],
                              in_=pt_oh_t[:, ch0 * P:(ch0 + n_chunks) * P])
            # a1vw for this tile: a1vw[0:32, t*64 + j*2nh] = a1v[32j:32j+32]
            # via 4 PE select-matmuls (lhsT = identity column slices)
            auxw = auxp.tile([P, 2 * P], F32, tag="aux", space="PSUM")
            for j in range(NG):
                nc.tensor.matmul(out=auxw[0:WIN, j * 2 * H:j * 2 * H + 2 * nhl],
                                 lhsT=eyeb[:, j * WIN:(j + 1) * WIN],
                                 rhs=a1v[:, t * 2 * H:t * 2 * H + 2 * nhl],
                                 start=True, stop=True)
            nc.vector.tensor_copy(
                out=a1vw[:, t * NG * 2 * H:t * NG * 2 * H + NG * 2 * H],
                in_=auxw[0:WIN, 0:NG * 2 * H])
            ci_tile = 0
            for nb in batches_t[t]:
                ni = nb * P
                g8 = g8pool.tile([P, CPB, rowW], BF16,
                                 tag="g8f" if final else "g8")
                nc.gpsimd.dma_gather(
                    g8[:, :nb, :], tb[:],
                    idxs[:, offs["idx"]:offs["idx"] + ni // 16],
                    ni, ni, rowW)
                offs["idx"] += ni // 16
                # --- a1 expansion (per chunk): one matmul gives hi||lo at
                # aux[e, ci*2H : ci*2H+2*nhl]; hi+lo summed in the chain
                aux = auxp.tile([P, CPB * 2 * H], F32, tag="aux", space="PSUM")
                psum_zero(aux, CPB * 2 * H)
                for ci in range(nb):
                    j = chunk_list[ci_tile + ci][0]
                    ptm = pball[:, (ci_tile + ci) * P:(ci_tile + ci + 1) * P]
                    rbw = t * NG * 2 * H + j * 2 * H
                    nc.tensor.matmul(
                        out=aux[:, ci * 2 * H:ci * 2 * H + 2 * nhl], lhsT=ptm,
                        rhs=a1vw[:, rbw:rbw + 2 * nhl],
                        start=False, stop=(ci == nb - 1))
                # --- per-edge weight chain (batched) ---
                smw = nb * H if not final else nb
                aux_v = aux[:, 0:nb * 2 * H].rearrange("p (c x) -> p c x",
                                                       x=2 * H)
                if final:
                    aux_hi = aux_v[:, :, 0]
                    aux_lo = aux_v[:, :, 1]
                    a2hi_s = g8[:, :nb, F1]
                    a2lo_s = g8[:, :nb, F1 + 1]
                else:
                    aux_hi = aux_v[:, :, 0:nhl]
                    aux_lo = aux_v[:, :, nhl:2 * nhl]
                    send = (nhl - 1) * blk + 1
                    a2hi_s = g8[:, :nb, F1:F1 + send:blk]
                    a2lo_s = g8[:, :nb, F1 + 1:F1 + 1 + send:blk]
                sm = spool.tile([P, CPB * H], F32, tag="sme")
                e1 = spool.tile([P, CPB * H], BF16, tag="sme2")
                e2 = spool.tile([P, CPB * H], BF16, tag="sme3")
                wpb = spool.tile([P, CPB * H], BF16, tag="wpb")
                tt = sm[:, 0:smw]
                nc.vector.tensor_tensor(out=tt, in0=aux_hi, in1=a2hi_s,
                                        op=OP.add)
                nc.vector.tensor_tensor(out=tt, in0=aux_lo, in1=tt, op=OP.add)
                nc.vector.tensor_tensor(out=tt, in0=tt, in1=a2lo_s, op=OP.add)
                # exp(leakyrelu(s)) = max(exp(s), exp(0.01*s)) — ScalarE
                # stays on the Exp table (no ACT_TABLE_LOAD switches)
                nc.scalar.activation(out=e1[:, 0:smw], in_=tt, func=AF.Exp)
                nc.scalar.activation(out=e2[:, 0:smw], in_=tt, func=AF.Exp,
                                     scale=SLOPE)
                nc.vector.tensor_tensor(out=wpb[:, 0:smw], in0=e1[:, 0:smw],
                                        in1=e2[:, 0:smw], op=OP.max)
                # --- S' build (groups of 4 chunks) ---
                spws = {}
                for g0 in range(0, nb, 4):
                    gn = min(4, nb - g0)
                    spw = wpool.tile([P, 4 * nhl, WIN], BF16, tag="sp")
                    spws[g0] = spw
                    if final:
                        w_sl = wpb[:, g0:g0 + gn, None]
                    else:
                        w_sl = wpb[:, g0 * H:(g0 + gn) * H, None]
                    mm = mball[:, (ci_tile + g0) * WIN:
                               (ci_tile + g0 + gn) * WIN]
                    nc.vector.tensor_tensor(
                        out=spw[:, 0:gn * nhl, :].rearrange(
                            "p (c h) d -> p c h d", h=nhl),
                        in0=mm.rearrange("p (c d) -> p c d", d=WIN)[
                            :, :, None, :].broadcast_to([P, gn, nhl, WIN]),
                        in1=w_sl.rearrange("p (c h) o -> p c h o",
                                           h=nhl).broadcast_to([P, gn, nhl, WIN]),
                        op=OP.mult)
                # --- aggregation matmuls ---
                for ci in range(nb):
                    j, k = chunk_list[ci_tile + ci]
                    first = (ci_tile + ci == 0)
                    last = (ci_tile + ci == n_chunks - 1)
                    gj_first = (k == 0)
                    gj_last = (k == nch[t][j] - 1)
                    spw = spws[(ci // 4) * 4]
                    cio = ci % 4
                    for h in range(nhl):
                        if final:
                            sto = last
                        else:
                            sto = last and h in (2, 5, 7)
                        nc.tensor.matmul(
                            out=acc_ap(h, (j * WIN, j * WIN + WIN)),
                            lhsT=spw[:, cio * nhl + h, :],
                            rhs=g8[:, ci, h * blk:h * blk + F1],
                            start=False, stop=sto,
                            tile_position=(0, j * WIN))
                ci_tile += nb
            # ---- finalize ----
            den = spool.tile([P, H], F32, tag="den")
            rec = spool.tile([P, H], F32, tag="rec")
            if final:
                nc.vector.tensor_copy(out=den[:rows, 0:1],
                                      in_=accC[:rows, F:F + 1])
            else:
                nc.vector.tensor_copy(
                    out=den[:rows, 0:3],
                    in_=accA[:rows].rearrange("p (h b) -> p h b", b=F1)[:, :, F])
                nc.vector.tensor_copy(
                    out=den[:rows, 3:6],
                    in_=accB[:rows].rearrange("p (h b) -> p h b", b=F1)[:, :, F])
                nc.vector.tensor_copy(
                    out=den[:rows, 6:8],
                    in_=accC[:rows, :2 * F1].rearrange(
                        "p (h b) -> p h b", b=F1)[:, :, F])
            nc.vector.reciprocal(out=rec[:rows, :nhl], in_=den[:rows, :nhl])
            fdt = F32 if final else BF16
            xw = nhl * F
            xo = rpool.tile([P, xw], fdt, tag="xof" if final else "xo")
            mn = rpool.tile([P, xw], fdt, tag="mnf" if final else "mn")
            # e = exp(acc*rec) per head (ScalarE, Exp only); xon = acc*rec on
            # DVE per bank; elu = max(xon, min(e,1) - 1)
            for h in range(nhl):
                nc.scalar.activation(out=mn[:rows, h * F:(h + 1) * F],
                                     in_=acc_ap(h, (0, rows))[:, 0:F],
                                     func=AF.Exp, scale=rec[:rows, h:h + 1])
            if final:
                nc.vector.tensor_tensor(
                    out=xo[:rows, 0:F], in0=accC[:rows, 0:F],
                    in1=rec[:rows, 0:1].broadcast_to([rows, F]), op=OP.mult)
            else:
                for bi, (base, h0, nh_) in enumerate(
                        [(accA, 0, 3), (accB, 3, 3), (accC, 6, 2)]):
                    nc.vector.tensor_tensor(
                        out=xo[:rows, h0 * F:(h0 + nh_) * F].rearrange(
                            "p (h f) -> p h f", f=F),
                        in0=base[:rows, :nh_ * F1].rearrange(
                            "p (h f) -> p h f", f=F1)[:, :, 0:F],
                        in1=rec[:rows, h0:h0 + nh_, None].broadcast_to(
                            [rows, nh_, F]),
                        op=OP.mult)
            nc.vector.tensor_tensor(out=mn[:rows, :xw], in0=mn[:rows, :xw],
                                    in1=ones_t[:rows, :xw], op=OP.min)
            nc.vector.tensor_tensor(out=mn[:rows, :xw], in0=mn[:rows, :xw],
                                    in1=ones_t[:rows, :xw], op=OP.subtract)
            nc.vector.tensor_tensor(out=xo[:rows, :xw], in0=xo[:rows, :xw],
                                    in1=mn[:rows, :xw], op=OP.max)
            if final:
                nc.sync.dma_start(out=out_t[t * P:t * P + rows, :],
                                  in_=xo[:rows, 0:C])
            else:
                if cfg.debug and layer == 0:
                    nc.sync.dma_start(out=dbg["x1"][t * P:t * P + rows, :],
                                      in_=xo[:rows, :])
                for h in range(nhl):
                    aux2 = auxp.tile([P, 2 * P], BF16, tag="aux", space="PSUM")
                    nc.tensor.matmul(out=aux2[:, 0:P],
                                     lhsT=xo[:, h * F:(h + 1) * F],
                                     rhs=eyeb[:], is_transpose=True,
                                     start=True, stop=True)
                    nc.vector.tensor_copy(
                        out=xt_b[:, h * NSH + t * P:h * NSH + t * P + rows],
                        in_=aux2[:, 0:rows])

        # ============ main sequence ============
        for t in range(NT):
            dense_tile(0, t)
            if t % 4 == 3 or t == NT - 1:
                ag_group(0, t // 4)
        if cfg.debug:
            nc.sync.dma_start(out=dbg["tbl0"][:], in_=tbl[0][:])
        offs = dict(idx=0, ch=0)
        for t in range(NT):
            edge_tile(0, t, offs)
            dense_tile(1, t)
            if t % 4 == 3 or t == NT - 1:
                ag_group(1, t // 4)
        offs = dict(idx=0, ch=0)
        for t in range(NT):
            edge_tile(1, t, offs)
            dense_tile(2, t)
            if t % 4 == 3 or t == NT - 1:
                ag_group(2, t // 4)
        offs = dict(idx=0, ch=0)
        for t in range(NT):
            edge_tile(2, t, offs)

    nc.compile()
    return nc


# ======================= runner =======================
_CACHE = {}


def _install_profhook():
    import ctypes
    import sys
    import types
    if "antenv.axon_hooks" in sys.modules:
        return
    so_path = "/opt/axon/libaxon_pjrt.so"
    mod = types.ModuleType("antenv.axon_hooks")
    state = {"hook": None}
    mod.set_axon_ntff_profile_hook = lambda h: state.__setitem__("hook", h)
    mod.get_axon_ntff_profile_hook = lambda: state["hook"]
    sys.modules["antenv.axon_hooks"] = mod
    try:
        import antenv
        antenv.axon_hooks = mod
        lib = ctypes.CDLL(so_path)
        if hasattr(lib, "axon_start_nrt_profile"):
            from trn_agent_boot.trn_boot import _ntff_profile_via_ctypes
            mod.set_axon_ntff_profile_hook(_ntff_profile_via_ctypes(so_path))
    except Exception:
        pass


def _kernel_impl(inputs, trace=False, debug=False):
    from concourse.bass_utils import run_bass_kernel_spmd
    if trace:
        _install_profhook()
    cfg = Cfg(debug=debug)
    in_maps, meta = host_prep(cfg, inputs)
    key = f"nc{debug}"
    if key not in _CACHE:
        _CACHE[key] = build_nc(cfg, meta)
    nc = _CACHE[key]
    res = run_bass_kernel_spmd(nc, in_maps, core_ids=list(range(cfg.NC)),
                               trace=trace)
    out = np.concatenate([res.results[c]["out"] for c in range(cfg.NC)],
                         axis=0)
    return out, res


def kernel(**inputs) -> np.ndarray:
    out, _ = _kernel_impl(inputs, trace=False)
    return out
